# revision 13
# baseline (speedup 1.0000x reference)
"""Chamfer distance loss on Trainium2 (Bass/Tile), 8-core SPMD — v2.

Math per batch b (inp/tgt: (B, C, N), mask: (B, N)):
    x = inp[b].T * mask[b,:,None]   # (N, 3)
    y = tgt[b].T * mask[b,:,None]
    d[n,m] = ||x_n||^2 + ||y_m||^2 - 2 x_n.y_m
    loss   = mean(min_m d) + mean(min_n d)     (means over all B*N)

v2 design (vs the 584us baseline):
  * K=5 augmented f32r matmuls produce the full NEGATED distance matrix
    e = -d directly (norm terms folded into the matmul), in both layouts:
    pass A tiles are (n x m), pass B tiles are (m x n).  All reductions
    become MAX ops (min d = -max e), which every engine op supports.
  * Row maxes (the free axis) run on DVE as fused tensor_tensor_reduce
    (max elementwise of two adjacent 1024-column PSUM quarters, then max
    along the free axis) - one op covers 2048 columns.
  * Column maxes run on Pool (gpsimd) as partition_all_reduce(max) over
    the first MP columns of every pass-A tile; this REPLACES the first
    MP//128 pass-B tiles entirely, cutting PE and DVE work.  Per-tile
    results are DMA-hopped (idle SP engine + DMA) onto the partitions of
    a [32, MP] stack, and a second partition_all_reduce finishes the
    column direction per batch.
  * PSUM is one [128, 4096] region used as 4 rotating 1024-column
    quarters, so the PE never write-after-read stalls on a consumer.
  * ACT does all the operand prep (x2/y2 norm rows via Square, the
    2*masked moving-operand copies, norm/one row broadcasts), keeping
    DVE/Pool for the N^2 reductions.

Host: shard batches across 8 cores (2 each), run SPMD, sum the per-core
partial sums, negate, divide by B*N.
"""

import numpy as np

B, C, N = 16, 3, 4096
NCORES = 8
BPC = B // NCORES        # batches per core
NT = N // 128            # 32 tiles per pass
QW = 1024                # PSUM quarter width (f32)
MPT = 16                 # pass-B tiles replaced by Pool column-reduction
MP = 128 * MPT           # Pool-covered m-columns (from pass-A tiles)
BIG = float(np.finfo(np.float32).max)

_CACHE = {}


def _build():
    from contextlib import ExitStack

    from concourse import bacc, bass, bass_isa, mybir, tile  # noqa: F401

    f32 = mybir.dt.float32
    f32r = mybir.dt.float32r
    bf16 = mybir.dt.bfloat16
    Alu = mybir.AluOpType
    Act = mybir.ActivationFunctionType
    RO = bass_isa.ReduceOp

    nc = bacc.Bacc(trn_type="TRN2", target_bir_lowering=False, debug=False)

    inp_d = nc.dram_tensor("inp", [BPC, C, N], f32, kind="ExternalInput").ap()
    tgt_d = nc.dram_tensor("tgt", [BPC, C, N], f32, kind="ExternalInput").ap()
    mask_d = nc.dram_tensor("mask", [BPC, N], f32, kind="ExternalInput").ap()
    # row 0 = +1, row 1 = -1 (host-provided constants; f32r so the row
    # DMAs into the operand tiles are cast-free)
    ones_d = nc.dram_tensor("ones", [2, N], f32r, kind="ExternalInput").ap()
    # col 0: per-partition row-direction sums; [0,1]: Pool column sums.
    out_d = nc.dram_tensor("out", [128, 2], f32, kind="ExternalOutput").ap()

    # Units: (group, tile).  Groups at partition 32g: g0=(A,b0) lhsT=x(b0),
    # g1=(B,b0) lhsT=y(b0), g2=(A,b1), g3=(B,b1).  Pass-B tiles below MPT
    # are covered by the Pool column pass instead.  A (Pool-fed) and B
    # (DVE-only) units are interleaved evenly so both reducers stay busy.
    a_units = []
    b_units = []
    for k in range(NT // 2):
        a_units.append((0, 2 * k))
        a_units.append((0, 2 * k + 1))
        a_units.append((2, 2 * k))
        a_units.append((2, 2 * k + 1))
    for t in range(MPT, NT):
        b_units.append((1, t))
        b_units.append((3, t))
    units = []
    bi = 0
    for i, a in enumerate(a_units):
        units.append(a)
        want = ((i + 1) * len(b_units)) // len(a_units)
        while bi < want:
            units.append(b_units[bi])
            bi += 1
    units.extend(b_units[bi:])
    U = len(units)

    with tile.TileContext(nc) as tc, ExitStack() as ctx:
        pool = ctx.enter_context(tc.tile_pool(name="main", bufs=1))

        wtr = pool.tile([128, N], f32r)   # stationary: coords/norm/one rows
        rtr = pool.tile([128, N], f32r)   # moving: 2*coords/-1/-norm rows
        psum = ctx.enter_context(
            tc.tile_pool(name="ps", bufs=1, space="PSUM")
        ).tile([128, N], f32)
        dcols = pool.tile([128, 2 * U], f32)   # ttr accums (2 per unit)
        stack = [pool.tile([NT, MP], bf16, name=f"stack{b}") for b in range(BPC)]
        pout = pool.tile([1, BPC + 1], f32)
        dtot = pool.tile([128, 1], f32)
        dmax = pool.tile([128, U], f32)

        with tc.tile_pool(name="prep", bufs=1) as prpool:
            wraw = prpool.tile([128, N], f32)  # raw coords (rows 32g..+3)
            mr = prpool.tile([128, N], f32)    # broadcast mask rows
            n4g = [prpool.tile([1, N], f32r, name=f"n4g{g}") for g in range(4)]
            sqz = [prpool.tile([3, N], f32, name=f"sqz{g}") for g in range(4)]

            # All input DMAs dispatch first (split over the idle SP and
            # Pool sequencers) so no load queues behind a compute-waiting
            # op.  Then per group: mask-mul (DVE, the only prep DVE work),
            # squares of the 3 coord rows (ACT, into the dead wraw rows),
            # 3-channel partition-add (Pool, into the dead mr rows), a
            # rounding copy (ACT, ordered after all squares/muls so its
            # Pool wait never head-of-line blocks them), then DMAs place
            # the aug rows.
            dsp = [nc.sync, nc.gpsimd]
            for g in range(4):
                p, i = g // 2, g % 2
                src = inp_d[p] if i == 0 else tgt_d[p]
                dsp[i].dma_start(out=wraw[32 * g : 32 * g + 3, :], in_=src)
                dsp[i].dma_start(
                    out=mr[32 * g : 32 * g + 3, :],
                    in_=mask_d[p : p + 1, :].broadcast_to((3, N)),
                )
            for pp in range(2):
                for g in (2 * pp, 2 * pp + 1):
                    r = slice(32 * g, 32 * g + 3)
                    nc.vector.tensor_mul(wtr[r, :], wraw[r, :], mr[r, :])
                    nc.scalar.activation(wraw[r, :], wtr[r, :], Act.Square)
                    # partition_all_reduce only works from partition 0 on
                    # real HW: hop the squared rows down first
                    dsp[g % 2].dma_start(out=sqz[g][:], in_=wraw[r, :])
                    nc.gpsimd.partition_all_reduce(
                        sqz[g][:], sqz[g][:], 3, RO.add
                    )
                # Moving-side data rows on the (startup-idle) DVE:
                # rtr rows0-2 = 2 * other-group coords.
                for g in (2 * pp, 2 * pp + 1):
                    o = g ^ 1
                    nc.vector.tensor_scalar_mul(
                        rtr[32 * g : 32 * g + 3, :],
                        wtr[32 * o : 32 * o + 3, :],
                        2.0,
                    )
            # Aug norm/const rows (e = 2x.y - x2 - y2, all via + rows and
            # two -1 constants): wtr row3 = own norm, row4 = -1; rtr
            # row3 = -1, row4 = other norm.
            for g in range(4):
                nc.scalar.copy(n4g[g][:], sqz[g][0:1, :])
            for g in range(4):
                i = g % 2
                o = g ^ 1
                dsp[i].dma_start(
                    out=wtr[32 * g + 3 : 32 * g + 4, :], in_=n4g[g][:]
                )
                dsp[i].dma_start(
                    out=rtr[32 * g + 4 : 32 * g + 5, :], in_=n4g[o][:]
                )
                dsp[i].dma_start(
                    out=wtr[32 * g + 4 : 32 * g + 5, :], in_=ones_d[1:2, :]
                )
                dsp[i].dma_start(
                    out=rtr[32 * g + 3 : 32 * g + 4, :], in_=ones_d[1:2, :]
                )

        spool = ctx.enter_context(tc.tile_pool(name="scr", bufs=4))
        parpool = ctx.enter_context(tc.tile_pool(name="par", bufs=3))
        parcpool = ctx.enter_context(tc.tile_pool(name="parc", bufs=4))
        parc2pool = ctx.enter_context(tc.tile_pool(name="parc2", bufs=4))
        scbpool = ctx.enter_context(tc.tile_pool(name="scb", bufs=3))

        # Quarter stream: each (g, t) unit is four 1024-column quarters;
        # quarter k always lands in PSUM slot k (depth-4 pipeline).  Every
        # PSUM range has exactly ONE reader so each matmul carries at most
        # one semaphore wait (multi-wait joins head-of-line block the PE
        # sequencer):
        #   A units: q0/q1 are read only by ACT copies into bf16 SBUF;
        #     DVE's first row-max ttr and Pool's column maxes both consume
        #     the copy.  q2/q3 are read only by the second row-max ttr.
        #   B units: both ttrs read PSUM directly.
        # Pool results for a PAIR of same-batch A-units collect (as bf16)
        # in one par tile, then one DMA hop moves both rows onto the
        # batch stack.
        par = None
        a_idx = 0
        for u, (g, t) in enumerate(units):
            gp = slice(32 * g, 32 * g + 5)
            lhsT = wtr[gp, t * 128 : (t + 1) * 128]
            isa = g % 2 == 0
            if isa and a_idx % 2 == 0:
                par = parpool.tile([128, 2 * MP], bf16, tag="par", name="par")
            off = (a_idx % 2) * MP
            if isa:
                parc = parcpool.tile([128, MP], bf16, tag="parc", name="parc")
                parc2 = parc2pool.tile([128, QW], bf16, tag="parc2", name="parc2")
            # A units emit the PSUM-direct half (q2/q3) BEFORE the q0/q1
            # copy chain, so the DVE always has ready work in front of the
            # copy-dependent ops.  Row maxes use tensor_tensor_scan
            # (max,max) — the fused 2-input reduce; its last column is the
            # row max (tensor_tensor_reduce does not run on this HW).  At
            # most one scan input may be PSUM, so q2 is ACT-copied.
            qorder = (2, 3, 0, 1) if isa else (0, 1, 2, 3)
            for q in qorder:
                sb = q * QW              # PSUM slot base == m-column base
                for j in range(2):
                    nc.tensor.matmul(
                        psum[:, sb + j * 512 : sb + (j + 1) * 512],
                        lhsT,
                        rtr[gp, sb + j * 512 : sb + (j + 1) * 512],
                        start=True,
                        stop=True,
                        tile_position=(32 * g, 0),
                    )
                if isa and q == 2:
                    # sole PSUM reader of q2: copy to bf16 SBUF
                    nc.scalar.copy(parc2[:], psum[:, sb : sb + QW])
                elif isa and q == 3:
                    # row max of the q2/q3 half: scan(q3 PSUM, q2 copy)
                    sc = spool.tile([128, QW], bf16, tag="sc", name="sc")
                    nc.vector.tensor_tensor_scan(
                        out=sc[:],
                        data0=psum[:, sb : sb + QW],
                        data1=parc2[:],
                        initial=-BIG,
                        op0=Alu.max,
                        op1=Alu.max,
                    )
                    nc.vector.tensor_copy(
                        dcols[:, 2 * u + 1 : 2 * u + 2], sc[:, QW - 1 : QW]
                    )
                elif isa and q < 2:
                    # sole PSUM reader of q0/q1: copy to bf16 SBUF
                    nc.scalar.copy(parc[:, sb : sb + QW], psum[:, sb : sb + QW])
                    if q == 1:
                        # row max of the copied half: all-SBUF scan
                        sc = spool.tile([128, QW], bf16, tag="sc", name="sc")
                        nc.vector.tensor_tensor_scan(
                            out=sc[:],
                            data0=parc[:, 0:QW],
                            data1=parc[:, QW : 2 * QW],
                            initial=-BIG,
                            op0=Alu.max,
                            op1=Alu.max,
                        )
                        nc.vector.tensor_copy(
                            dcols[:, 2 * u : 2 * u + 1], sc[:, QW - 1 : QW]
                        )
                        # column maxes of the copied half
                        for j in range(2):
                            nc.gpsimd.partition_all_reduce(
                                par[:, off + j * QW : off + (j + 1) * QW],
                                parc[:, j * QW : (j + 1) * QW],
                                128,
                                RO.max,
                            )
                elif q % 2 == 0:
                    # B units, even quarter: sole PSUM reader is an ACT
                    # copy (the idle ACT has ample slack)
                    parc2 = parc2pool.tile(
                        [128, QW], bf16, tag="parc2", name="parc2"
                    )
                    nc.scalar.copy(parc2[:], psum[:, sb : sb + QW])
                else:
                    # B units, odd quarter: fused scan(PSUM, copy)
                    h = q // 2
                    sc = spool.tile([128, QW], bf16, tag="sc", name="sc")
                    nc.vector.tensor_tensor_scan(
                        out=sc[:],
                        data0=psum[:, sb : sb + QW],
                        data1=parc2[:],
                        initial=-BIG,
                        op0=Alu.max,
                        op1=Alu.max,
                    )
                    nc.vector.tensor_copy(
                        dcols[:, 2 * u + h : 2 * u + h + 1], sc[:, QW - 1 : QW]
                    )
            if isa:
                if a_idx % 2 == 1:
                    # pair (t-1, t) complete: hop both rows at once
                    nc.sync.dma_start(
                        out=stack[g // 2][t - 1 : t + 1, :],
                        in_=par[0:1, 0 : 2 * MP],
                    )
                a_idx += 1

        # Column-direction finish: per batch, partition max of the 32
        # stacked rows, then sum over the MP columns.
        finpool = ctx.enter_context(tc.tile_pool(name="finp", bufs=1))
        for b in range(BPC):
            fin = finpool.tile([128, MP], bf16, tag="fin", name="fin")
            nc.gpsimd.partition_all_reduce(fin[0:NT, :], stack[b][:], NT, RO.max)
            nc.vector.tensor_reduce(
                pout[0:1, b : b + 1],
                fin[0:1, :],
                axis=mybir.AxisListType.X,
                op=Alu.add,
            )

        # Row-direction finish: per-unit max of its two half-row maxes,
        # then sum across units per partition.
        nc.vector.tensor_reduce(
            dmax[:],
            dcols[:].rearrange("p (u two) -> p u two", two=2),
            axis=mybir.AxisListType.X,
            op=Alu.max,
        )
        nc.vector.tensor_reduce(
            dtot[:], dmax[:], axis=mybir.AxisListType.X, op=Alu.add
        )
        nc.vector.tensor_reduce(
            pout[0:1, BPC : BPC + 1],
            pout[0:1, 0:BPC],
            axis=mybir.AxisListType.X,
            op=Alu.add,
        )
        nc.sync.dma_start(out=out_d[:, 0:1], in_=dtot[:])
        nc.sync.dma_start(out=out_d[0:1, 1:2], in_=pout[0:1, BPC : BPC + 1])

    nc.compile()
    return nc


def _get_nc():
    if "nc" not in _CACHE:
        _CACHE["nc"] = _build()
    return _CACHE["nc"]


def _in_maps(inp, tgt, mask):
    inp = np.ascontiguousarray(inp, dtype=np.float32)
    tgt = np.ascontiguousarray(tgt, dtype=np.float32)
    mask = np.ascontiguousarray(mask, dtype=np.float32)
    ones = np.empty((2, N), dtype=np.float32)
    ones[0] = 1.0
    ones[1] = -1.0
    return [
        {
            "inp": inp[c * BPC : (c + 1) * BPC],
            "tgt": tgt[c * BPC : (c + 1) * BPC],
            "mask": mask[c * BPC : (c + 1) * BPC],
            "ones": ones,
        }
        for c in range(NCORES)
    ]


def _run(in_maps, **kwargs):
    from concourse.bass_utils import run_bass_kernel_spmd

    return run_bass_kernel_spmd(_get_nc(), in_maps, list(range(NCORES)), **kwargs)


def kernel(inp, tgt, mask):
    res = _run(_in_maps(inp, tgt, mask))
    total = 0.0
    for r in res.results:
        o = r["out"]
        total += float(o[:, 0].sum()) + float(o[0, 1])
    return np.float32(-total / (B * N))
